# revision 5
# baseline (speedup 1.0000x reference)
"""AttentionSequencePoolingLayer (DIN-style) kernel for Trainium2, 8 cores.

Reference, per batch row b (W = [Wq; Wk], each [64, 1]):
    score_t = tanh(keys_b[t] @ Wk + (query_b @ Wq + bias))
    logits  = where(t < keys_length_b, score_t, MASK_PAD)
    out_b   = softmax(logits) @ keys_b
Masking here: e = exp((score+30)*mask - 30); masked lanes give exp(-30),
which vanishes next to real weights and reproduces the reference's
uniform-softmax behaviour when every position is masked (keys_length==0).

Sharding: pure data parallel, batch 4096 -> 8 NeuronCores x 512.

Design, driven by measured TRN2 facts (this toolchain):
  - Only the natural [b, (t c)] keys DMA reaches full HBM rate (~320 GB/s);
    transposed layouts run 2-4x slower, which rules out every TensorE
    matmul formulation (PE contracts the partition dim = batch here).
  - So both contractions run on VectorE in bf16 (tensor_tensor at 2x with
    step-1 innermost APs) with pairwise fold trees replacing tensor_reduce
    (always 1x). A stride-0 operand drops TT to 1x, so e is pre-expanded
    along c on ScalarE, which also does the f32->bf16 keys convert, tanh,
    exp (with fused sum) and the 1/s scaling.
  - GpSimd runs nothing: its SBUF port is lock-shared with VectorE and its
    queue would serialize with DMAs.

Per 128-batch tile, pipelined across tiles by the Tile framework, in
t-chunks of (64, 64, 72) so DMA/ScalarE/VectorE interleave finely:
  sync DMA f32 chunk -> ACT convert to bf16 -> DVE prod = keys*Wk(bcast)
  -> DVE c-fold tree -> ACT tanh(kdot+qdot+b) -> mask/exp/normalize
  -> ACT expand e -> DVE p2 = keys*e -> DVE t-fold tree -> join -> DMA out.
"""

import sys

sys.path.insert(0, "/opt/trn_rl_repo")

import numpy as np

import concourse.bass as bass
import concourse.tile as tile
from concourse import bacc, mybir
from concourse.bass_utils import run_bass_kernel_spmd

F32 = mybir.dt.float32
BF16 = mybir.dt.bfloat16
I32 = mybir.dt.int32

B_FULL = 4096
N_CORES = 8
B = B_FULL // N_CORES  # 512
T = 200
C = 64
P = 128
N_TILES = B // P  # 4

_NC_CACHE = {}


def build_kernel():
    nc = bacc.Bacc("TRN2", target_bir_lowering=False, debug=False)

    q_d = nc.dram_tensor("queries", [B, 1, C], F32, kind="ExternalInput").ap()
    k_d = nc.dram_tensor("keys", [B, T, C], F32, kind="ExternalInput").ap()
    kl_d = nc.dram_tensor("keys_length", [B, 1], I32, kind="ExternalInput").ap()
    w_d = nc.dram_tensor("W", [2 * C, 1], F32, kind="ExternalInput").ap()
    b_d = nc.dram_tensor("b", [1], F32, kind="ExternalInput").ap()
    out_d = nc.dram_tensor("out", [B, 1, C], F32, kind="ExternalOutput").ap()

    with tile.TileContext(nc) as tc:
        with (
            tc.tile_pool(name="const", bufs=1) as cpool,
            tc.tile_pool(name="kf32", bufs=2) as fpool,
            tc.tile_pool(name="keys", bufs=2) as kpool,
            tc.tile_pool(name="prod", bufs=1) as ppool,
            tc.tile_pool(name="p2p", bufs=1) as p2pool,
            tc.tile_pool(name="ex", bufs=1) as xpool,
            tc.tile_pool(name="small", bufs=2) as spool,
            tc.tile_pool(name="ps", bufs=1, space="PSUM") as ps,
        ):
            # ---- setup: broadcast W row + bias to all partitions ----
            wrow = cpool.tile([1, 2 * C + 1], F32)
            nc.sync.dma_start(wrow[:, 0 : 2 * C], w_d.rearrange("c o -> o c"))
            nc.sync.dma_start(wrow[:, 2 * C : 2 * C + 1], b_d.unsqueeze(0))
            ones_col = cpool.tile([1, P], F32)
            nc.vector.memset(ones_col[:], 1.0)
            wbc_ps = ps.tile([P, 2 * C + 1], F32)
            nc.tensor.matmul(wbc_ps[:], ones_col[:], wrow[:], start=True, stop=True)
            wbc = cpool.tile([P, 2 * C + 1], F32)
            nc.vector.tensor_copy(wbc[:], wbc_ps[:])
            wq_bc = wbc[:, 0:C]
            bias_bc = wbc[:, 2 * C : 2 * C + 1]
            wk_bf = cpool.tile([P, C], BF16)
            nc.vector.tensor_copy(wk_bf[:], wbc[:, C : 2 * C])

            iota_i = cpool.tile([P, T], I32)
            nc.gpsimd.iota(iota_i[:], pattern=[[1, T]], base=0, channel_multiplier=0)
            iota_f = cpool.tile([P, T], F32)
            nc.vector.tensor_copy(iota_f[:], iota_i[:])
            neg30 = cpool.tile([P, 1], F32)
            nc.vector.memset(neg30[:], -30.0)

            for i in range(N_TILES):
                sl = slice(i * P, (i + 1) * P)

                # keys: HWDGE f32 chunk loads + ACT bf16 convert
                CH = ((0, 64), (64, 128), (128, T))
                kbf = kpool.tile([P, T * C], BF16)
                k3 = kbf[:].rearrange("p (t c) -> p t c", t=T, c=C)
                prod = ppool.tile([P, T * C], BF16, tag="prod")
                p3 = prod[:].rearrange("p (t c) -> p t c", t=T, c=C)
                for t0, t1 in CH:
                    kfh = fpool.tile([P, 128 * C], F32, tag="kfh")
                    nc.sync.dma_start(
                        kfh[:, 0 : (t1 - t0) * C],
                        k_d[sl, t0:t1, :].rearrange("b t c -> b (t c)"),
                    )
                    nc.scalar.copy(
                        kbf[:, t0 * C : t1 * C], kfh[:, 0 : (t1 - t0) * C]
                    )
                    # scores: product + c-fold tree for this chunk
                    nc.vector.tensor_tensor(
                        p3[:, t0:t1, :],
                        k3[:, t0:t1, :],
                        wk_bf[:].unsqueeze(1).to_broadcast((P, t1 - t0, C)),
                        op=mybir.AluOpType.mult,
                    )
                    w_ = C // 2
                    while w_ >= 1:
                        nc.vector.tensor_tensor(
                            p3[:, t0:t1, 0:w_],
                            p3[:, t0:t1, 0:w_],
                            p3[:, t0:t1, w_ : 2 * w_],
                            op=mybir.AluOpType.add,
                        )
                        w_ //= 2

                q_t = spool.tile([P, C], F32)
                nc.sync.dma_start(q_t[:], q_d[sl, 0, :])
                kl_t = spool.tile([P, 1], I32)
                nc.sync.dma_start(kl_t[:], kl_d[sl])
                kl_f = spool.tile([P, 1], F32)
                nc.vector.tensor_copy(kl_f[:], kl_t[:])

                # compact kdot (stride 64 -> dense f32) on ACT
                kdot = spool.tile([P, T], F32)
                for t0, t1 in CH:
                    nc.scalar.copy(kdot[:, t0:t1], p3[:, t0:t1, 0])

                # qdot = sum(q * Wq) + bias
                qprod = spool.tile([P, C], F32)
                nc.vector.tensor_tensor(
                    qprod[:], q_t[:], wq_bc, op=mybir.AluOpType.mult
                )
                qdot = spool.tile([P, 1], F32)
                nc.vector.reduce_sum(qdot[:], qprod[:], axis=mybir.AxisListType.X)
                qdotb = spool.tile([P, 1], F32)
                nc.vector.tensor_tensor(
                    qdotb[:], qdot[:], bias_bc, op=mybir.AluOpType.add
                )

                score = spool.tile([P, T], F32)
                nc.scalar.activation(
                    score[:],
                    kdot[:],
                    mybir.ActivationFunctionType.Tanh,
                    bias=qdotb[:],
                    scale=1.0,
                )
                mask = spool.tile([P, T], F32)
                nc.vector.tensor_scalar(
                    mask[:], iota_f[:], kl_f[:], None, op0=mybir.AluOpType.is_lt
                )
                sm = spool.tile([P, T], F32)
                nc.vector.scalar_tensor_tensor(
                    sm[:],
                    score[:],
                    30.0,
                    mask[:],
                    op0=mybir.AluOpType.add,
                    op1=mybir.AluOpType.mult,
                )
                e = spool.tile([P, T], F32)
                ssum = spool.tile([P, 1], F32)
                nc.scalar.activation(
                    e[:],
                    sm[:],
                    mybir.ActivationFunctionType.Exp,
                    bias=neg30[:],
                    scale=1.0,
                    accum_out=ssum[:],
                )
                rs = spool.tile([P, 1], F32)
                nc.vector.reciprocal(rs[:], ssum[:])
                # en = e / s (bf16), via ACT with per-partition scale
                en = spool.tile([P, T], BF16)
                nc.scalar.activation(
                    en[:],
                    e[:],
                    mybir.ActivationFunctionType.Copy,
                    bias=0.0,
                    scale=rs[:],
                )

                # ---- output: per chunk expand -> multiply -> t-fold ----
                enx = xpool.tile([P, T * C], BF16, tag="enx")
                enx3 = enx[:].rearrange("p (t c) -> p t c", t=T, c=C)
                en3 = en[:].unsqueeze(2).to_broadcast((P, T, C))
                p2 = p2pool.tile([P, T * C], BF16, tag="p2")
                p23 = p2[:].rearrange("p (t c) -> p t c", t=T, c=C)
                for t0, t1 in CH:
                    nc.scalar.copy(enx3[:, t0:t1, :], en3[:, t0:t1, :])
                    nc.vector.tensor_tensor(
                        p23[:, t0:t1, :],
                        k3[:, t0:t1, :],
                        enx3[:, t0:t1, :],
                        op=mybir.AluOpType.mult,
                    )
                    if t1 - t0 == 72:
                        nc.vector.tensor_tensor(
                            p23[:, t0 : t0 + 8, :],
                            p23[:, t0 : t0 + 8, :],
                            p23[:, t0 + 64 : t1, :],
                            op=mybir.AluOpType.add,
                        )
                    w_ = (t1 - t0) // 2 if t1 - t0 in (64, 128) else 32
                    while w_ >= 1:
                        nc.vector.tensor_tensor(
                            p23[:, t0 : t0 + w_, :],
                            p23[:, t0 : t0 + w_, :],
                            p23[:, t0 + w_ : t0 + 2 * w_, :],
                            op=mybir.AluOpType.add,
                        )
                        w_ //= 2
                outp = spool.tile([P, C], F32)
                nc.vector.tensor_tensor(
                    outp[:], p23[:, 0, :], p23[:, 64, :], op=mybir.AluOpType.add
                )
                out_t = spool.tile([P, C], F32)
                nc.vector.tensor_tensor(
                    out_t[:], outp[:], p23[:, 128, :], op=mybir.AluOpType.add
                )
                nc.sync.dma_start(out_d[sl, 0, :], out_t[:])

    nc.compile()
    return nc


def get_kernel():
    if "nc" not in _NC_CACHE:
        _NC_CACHE["nc"] = build_kernel()
    return _NC_CACHE["nc"]


def kernel(queries, keys, keys_length, W, b, **run_kwargs):
    nc = get_kernel()
    in_maps = []
    for c in range(N_CORES):
        sl = slice(c * B, (c + 1) * B)
        in_maps.append(
            {
                "queries": np.ascontiguousarray(queries[sl], dtype=np.float32),
                "keys": np.ascontiguousarray(keys[sl], dtype=np.float32),
                "keys_length": np.ascontiguousarray(keys_length[sl], dtype=np.int32),
                "W": np.ascontiguousarray(W, dtype=np.float32),
                "b": np.ascontiguousarray(b, dtype=np.float32),
            }
        )
    res = run_bass_kernel_spmd(nc, in_maps, core_ids=list(range(N_CORES)), **run_kwargs)
    out = np.concatenate([res.results[c]["out"] for c in range(N_CORES)], axis=0)
    if run_kwargs:
        kernel.last_result = res
    return out
